# revision 29
# baseline (speedup 1.0000x reference)
import sys

sys.path.insert(0, "/opt/trn_rl_repo")

from collections import deque
from contextlib import ExitStack

import numpy as np
import ml_dtypes

import concourse.bass as bass
import concourse.tile as tile
from concourse import bacc
from concourse import mybir

B, T, C = 4, 2048, 1024
NH, D = 16, 64
NCORES = 8
# core c owns batch c//2 and head-group c%2 (8 heads = 4 pairs of 2)
NP = 4                      # head pairs per core
F = NP * 2 * D              # per-core feature slice (512)
P = 128
TCH = 512                   # token chunk for qkv phase
QCH = 512                   # q chunk in attention
KT = 128                    # k tile in attention
f32 = mybir.dt.float32
f32r = mybir.dt.float32r
bf16 = mybir.dt.bfloat16
AF = mybir.ActivationFunctionType
NPBF = ml_dtypes.bfloat16


def build_nc(reps=1):
    """One-core SPMD program: this core owns one batch and 8 heads (4 pairs)."""
    t = T
    n_cc = C // P               # contraction chunks for qkv (8)
    n_tch = t // TCH            # token chunks (4)
    n_jj = t // QCH             # q chunks (4)
    n_kt_b = t // KT            # k tiles (16)

    nc = bacc.Bacc(None, target_bir_lowering=False)
    xT = nc.declare_dram_parameter("xT", [C, t], bf16, isOutput=False)
    wq = nc.declare_dram_parameter("wq", [C, F], bf16, isOutput=False)
    wk = nc.declare_dram_parameter("wk", [C, F], bf16, isOutput=False)
    wv = nc.declare_dram_parameter("wv", [C, F], bf16, isOutput=False)
    wo = nc.declare_dram_parameter("wo", [F, C], bf16, isOutput=False)
    cos2 = nc.declare_dram_parameter("cos2", [P, t], bf16, isOutput=False)
    sinS = nc.declare_dram_parameter("sinS", [P, t], bf16, isOutput=False)
    maskn = nc.declare_dram_parameter("maskn", [P, P], bf16, isOutput=False)
    ident = nc.declare_dram_parameter("ident", [P, P], bf16, isOutput=False)
    y = nc.declare_dram_parameter("y", [t, C], f32, isOutput=True)

    xT_t = xT.rearrange("(o p) n -> p o n", p=P)   # [128, 8, t]

    with tile.TileContext(nc) as tc, ExitStack() as ctx:
        consts = ctx.enter_context(tc.tile_pool(name="consts", bufs=1))
        xpool = ctx.enter_context(tc.tile_pool(name="xt", bufs=3))
        rope = ctx.enter_context(tc.tile_pool(name="rope", bufs=3))
        ppool = ctx.enter_context(tc.tile_pool(name="pp", bufs=4))
        otp = ctx.enter_context(tc.tile_pool(name="otp", bufs=3))
        ystage = ctx.enter_context(tc.tile_pool(name="yst", bufs=3))
        ps_s = ctx.enter_context(tc.tile_pool(name="pss", bufs=2, space="PSUM"))
        ps_o = ctx.enter_context(tc.tile_pool(name="pso", bufs=1, space="PSUM"))
        ps_q = ctx.enter_context(tc.tile_pool(name="psq", bufs=2, space="PSUM"))

        # resident constants (wq first, split per-cc: the first matmul only
        # needs cc=0, so start computing after ~1/8 of the load)
        wq_r = wq.rearrange("(o p) f -> p o f", p=P)
        wq_sb = consts.tile([P, n_cc, F], bf16)
        for h in range(2):
            nc.sync.dma_start(wq_sb[:, 4 * h : 4 * h + 4], wq_r[:, 4 * h : 4 * h + 4])
        wk_r = wk.rearrange("(o p) f -> p o f", p=P)
        wk_sb = consts.tile([P, n_cc, F], bf16)
        for h in range(2):
            nc.scalar.dma_start(wk_sb[:, 4 * h : 4 * h + 4], wk_r[:, 4 * h : 4 * h + 4])
        wv_r = wv.rearrange("(o p) f -> p o f", p=P)
        wv_sb = consts.tile([P, n_cc, F], bf16)
        for h in range(2):
            nc.scalar.dma_start(wv_sb[:, 4 * h : 4 * h + 4], wv_r[:, 4 * h : 4 * h + 4])
        cos_sb = consts.tile([P, t], bf16)
        nc.scalar.dma_start(cos_sb, cos2[:, :])
        sin_sb = consts.tile([P, t], bf16)
        nc.scalar.dma_start(sin_sb, sinS[:, :])
        wo_sb = consts.tile([P, NP, C], bf16)
        nc.scalar.dma_start(wo_sb, wo.rearrange("(r p) c -> p r c", p=P))
        mask_sb = consts.tile([P, P], bf16)
        nc.scalar.dma_start(mask_sb, maskn[:, :])
        id_sb = consts.tile([P, P], bf16)
        nc.scalar.dma_start(id_sb, ident[:, :])

        # persistent per-pair tensors
        q_sb = consts.tile([P, NP, t], bf16, name="q_sb")
        k_sb = consts.tile([P, NP, t], bf16, name="k_sb")
        v1_sb = consts.tile([P, NP, n_kt_b, 2, D + 1], bf16, name="v1_sb")
        nc.vector.memset(v1_sb[:, :, :, :, D], 1.0)
        # attention output staging (all jj live): [feat, jj, pair, tok]
        ot_sb = consts.tile([P, n_jj, NP, QCH], bf16, name="ot_sb")

        H = D // 2  # 32

        def rope_apply(ps, dst, c0, bulk_pool=False):
            """dst(bf16) = ps*cos + swap32(ps)*sin, feature-major [128, TCH].

            psum reads must be DVE/ACT (GPSIMD cannot access PSUM); the final
            all-SBUF add goes to Pool to keep DVE lean."""
            cos_sl = cos_sb[:, c0 : c0 + TCH]
            sin_sl = sin_sb[:, c0 : c0 + TCH]
            raw = rope.tile([P, TCH], bf16, tag="raw")
            nc.vector.tensor_copy(raw, ps)
            rot = rope.tile([P, TCH], bf16, tag="rot")
            for hh in range(2):
                b0 = hh * D
                nc.vector.tensor_copy(rot[b0 : b0 + H], raw[b0 + H : b0 + 2 * H])
                nc.vector.tensor_copy(rot[b0 + H : b0 + 2 * H], raw[b0 : b0 + H])
            mul_eng = nc.gpsimd if bulk_pool else nc.vector
            t1 = rope.tile([P, TCH], bf16, tag="t1")
            mul_eng.tensor_mul(out=t1, in0=raw, in1=cos_sl)
            t2 = rope.tile([P, TCH], bf16, tag="t2")
            mul_eng.tensor_mul(out=t2, in0=rot, in1=sin_sl)
            nc.gpsimd.tensor_add(out=dst, in0=t1, in1=t2)

        def qkv_chunk_bundles(it, first=False):
            """Emit-closures computing q/k/v for all pairs for token chunk it.

            All tiles are allocated lazily inside the closures so pool ring
            order matches actual emission order (bundles may be deferred)."""
            c0 = it * TCH
            bundles = []
            cell = {}
            def load(cc):
                def go():
                    if "xt" not in cell:
                        cell["xt"] = xpool.tile([P, n_cc, TCH], bf16, tag="xt", name="xt")
                    eng = nc.gpsimd if first else nc.sync
                    eng.dma_start(cell["xt"][:, cc], xT_t[:, cc, c0 : c0 + TCH])
                return go

            def load_half(h):
                # chunk 0 only: one DMA per half - fewer DGE generations at
                # the cold start
                def go():
                    if "xt" not in cell:
                        cell["xt"] = xpool.tile([P, n_cc, TCH], bf16, tag="xt", name="xt")
                    nc.gpsimd.dma_start(
                        cell["xt"][:, 4 * h : 4 * h + 4],
                        xT_t[:, 4 * h : 4 * h + 4, c0 : c0 + TCH],
                    )
                return go

            if first:
                bundles.append(load_half(0))
                bundles.append(load_half(1))
            else:
                for cc in range(n_cc):
                    bundles.append(load(cc))

            def alloc_ps(shape, key):
                # chunk 0 runs before attention: cycle its psum tiles through
                # the (idle) score pool too, doubling rotation depth
                if first:
                    n = cell.get("_pc", 0)
                    cell["_pc"] = n + 1
                    pool, tg = [(ps_q, "pq"), (ps_s, "mm")][n % 2]
                else:
                    pool, tg = ps_q, "pq"
                cell[key] = pool.tile(shape, f32, tag=tg, name=key)

            def qk_mms(w_sb, pair, key, lohi, alloc=False):
                def go():
                    if alloc:
                        alloc_ps([P, TCH], key)
                    ps = cell[key]
                    for cc in lohi:
                        nc.tensor.matmul(
                            ps,
                            lhsT=w_sb[:, cc, pair * P : (pair + 1) * P],
                            rhs=cell["xt"][:, cc, :],
                            start=(cc == 0),
                            stop=(cc == n_cc - 1),
                        )
                return go

            def rope_fin(key, dst_sb, pair):
                return lambda: rope_apply(cell[key], dst_sb[:, pair, c0 : c0 + TCH], c0)

            def v_tile(pair, s):
                def go():
                    key = f"psv{pair}"
                    if s == 0:
                        alloc_ps([P, TCH // P, 2, D], key)
                    ps = cell[key]
                    for cc in range(n_cc):
                        nc.tensor.matmul(
                            ps[:, s],
                            lhsT=cell["xt"][:, cc, s * P : (s + 1) * P],
                            rhs=wv_sb[:, cc, pair * P : (pair + 1) * P],
                            start=(cc == 0),
                            stop=(cc == n_cc - 1),
                        )
                    ti = it * (TCH // P) + s
                    nc.vector.tensor_copy(v1_sb[:, pair, ti, :, 0:D], ps[:, s])
                return go

            for pair in range(NP):
                kq, kk = f"psq{pair}", f"psk{pair}"
                bundles.append(qk_mms(wq_sb, pair, kq, range(0, 4), alloc=True))
                bundles.append(qk_mms(wq_sb, pair, kq, range(4, 8)))
                bundles.append(rope_fin(kq, q_sb, pair))
                bundles.append(qk_mms(wk_sb, pair, kk, range(0, 4), alloc=True))
                bundles.append(qk_mms(wk_sb, pair, kk, range(4, 8)))
                bundles.append(rope_fin(kk, k_sb, pair))
                for s in range(TCH // P):
                    bundles.append(v_tile(pair, s))
            return bundles

        def out_proj_bundles(jj, tail=False):
            """Emit-closures for the out-projection of q-chunk jj (all pairs)."""
            bundles = []

            def psy_tile(qt, nh, n):
                def go():
                    if tail:
                        # attention is over: borrow the idle score pool for a
                        # wider psY rotation, and put ysb copies on both DVE
                        # and the now-idle ACT
                        pool, tg = [(ps_q, "pq"), (ps_s, "mm")][n % 2]
                        psY = pool.tile([P, 512], f32, tag=tg, name="psY")
                    else:
                        psY = ps_q.tile([P, 512], f32, tag="pq", name="psY")
                    for pair in range(NP):
                        nc.tensor.matmul(
                            psY,
                            lhsT=ot_sb[:, jj, pair, qt * P : (qt + 1) * P],
                            rhs=wo_sb[:, pair, nh * 512 : (nh + 1) * 512],
                            start=(pair == 0),
                            stop=(pair == NP - 1),
                        )
                    ysb = ystage.tile([P, 512], f32, tag="ysb")
                    if tail and n % 2 == 1:
                        nc.scalar.copy(ysb, psY)
                    else:
                        nc.vector.tensor_copy(ysb, psY)
                    r0 = jj * QCH + qt * P
                    if tail:
                        deng = nc.sync if n % 2 == 0 else nc.gpsimd
                    else:
                        deng = nc.gpsimd
                    deng.dma_start(y[r0 : r0 + P, nh * 512 : (nh + 1) * 512], ysb)
                return go

            n = 0
            for qt in range(QCH // P):
                for nh in range(C // 512):
                    bundles.append(psy_tile(qt, nh, n))
                    n += 1
            return bundles

        filler = deque()          # entries: (chunk_idx_or_-1, closure)
        pending_chunk = {}
        # total fill() slots in the attention phase, for supply pacing:
        # per (jj, pair): (n_kt-1)*2 inside + 3 after the block
        pace = {"slots": sum(4 * ((4 * jj + 4 - 1) * 2 + 3) for jj in range(n_jj)),
                "acc": 0.0}

        def _pop():
            tag, fn = filler.popleft()
            if tag >= 0:
                pending_chunk[tag] -= 1
            fn()

        def fill(k):
            for _ in range(k):
                pace["slots"] -= 1
                if not filler:
                    continue
                # spread the available bundles over the remaining slots
                pace["acc"] += min(4.0, len(filler) / max(1, pace["slots"]))
                while filler and pace["acc"] >= 1.0:
                    pace["acc"] -= 1.0
                    _pop()

        def add_filler(tag, bundles):
            if tag >= 0:
                pending_chunk[tag] = pending_chunk.get(tag, 0) + len(bundles)
            filler.extend((tag, b) for b in bundles)

        def drain_chunk(cidx):
            while pending_chunk.get(cidx, 0) > 0:
                _pop()

        def emit_attn(pair, jj):
            n_kt = 4 * jj + 4
            psO = ps_o.tile([P, 2, QCH], f32, tag="o0")
            tiles = []

            def score(i):
                lo = max(0, i * KT - jj * QCH)
                qs = jj * QCH + lo
                psS = ps_s.tile([P, 2, QCH], f32, tag="mm")
                kt_sl = slice(i * KT, (i + 1) * KT)
                q_sl = slice(qs, (jj + 1) * QCH)
                for h in range(2):
                    nc.tensor.matmul(
                        psS[:, h, lo:],
                        lhsT=k_sb[h * D : (h + 1) * D, pair, kt_sl],
                        rhs=q_sb[h * D : (h + 1) * D, pair, q_sl],
                        start=True,
                        stop=True,
                    )
                Pp = ppool.tile([P, 2, QCH], bf16, tag="p0")
                nc.scalar.activation(Pp[:, :, lo:], psS[:, :, lo:], AF.Exp, scale=0.125)
                if i >= 4 * jj:  # diagonal tile: triangle mask both heads
                    nc.vector.tensor_mul(
                        Pp[:, :, lo : lo + P],
                        Pp[:, :, lo : lo + P],
                        mask_sb[:, None, :].to_broadcast((P, 2, P)),
                    )
                return lo, Pp

            def pv(i, lo, Pp):
                for h in range(2):
                    nc.tensor.matmul(
                        psO[0 : D + 1, h, lo:],
                        lhsT=v1_sb[:, pair, i, h, :],
                        rhs=Pp[:, h, lo:],
                        start=(i == 0),
                        stop=(i == n_kt - 1),
                    )

            for i in range(n_kt):
                tiles.append(score(i))
                if i > 0:
                    # filler goes between S(i) and PV(i-1): that's where the
                    # in-order PE queue would otherwise wait on exp(i-1)
                    fill(2)
                    pv(i - 1, *tiles[i - 1])
            pv(n_kt - 1, *tiles[n_kt - 1])

            # Copy psO out first (DVE: GPSIMD cannot read PSUM): frees the
            # psum bank for the next (pair, jj) block earlier than the
            # recip/mul chain would.
            oc = otp.tile([D + 1, 2, QCH], f32, tag="oc")
            nc.vector.tensor_copy(oc, psO[0 : D + 1, :, :])
            # softmax normalization: OT[h] = O[h] / l[h]; muls on Pool
            rs = otp.tile([1, 2, QCH], f32, tag="rs")
            nc.vector.reciprocal(rs, oc[D : D + 1, :, :])
            rb0 = otp.tile([D, QCH], f32, tag="rb0")
            nc.gpsimd.partition_broadcast(rb0, rs[0:1, 0, :])
            rb1 = otp.tile([D, QCH], f32, tag="rb1")
            nc.gpsimd.partition_broadcast(rb1, rs[0:1, 1, :])
            if jj == n_jj - 1 and pair == NP - 1:
                for qt in range(QCH // P):
                    qsl = slice(qt * P, (qt + 1) * P)
                    nc.vector.tensor_mul(ot_sb[0:D, jj, pair, qsl], oc[0:D, 0, qsl], rb0[:, qsl])
                    nc.vector.tensor_mul(ot_sb[D:P, jj, pair, qsl], oc[0:D, 1, qsl], rb1[:, qsl])
            else:
                nc.gpsimd.tensor_mul(ot_sb[0:D, jj, pair, :], oc[0:D, 0, :], rb0)
                nc.gpsimd.tensor_mul(ot_sb[D:P, jj, pair, :], oc[0:D, 1, :], rb1)

        # ---- emission schedule ----
        for rep in range(reps):
            # chunk 0 emitted directly; chunk jj+1 rides as filler inside
            # attn(*, jj) so attention (and ACT exp work) starts early
            for b in qkv_chunk_bundles(0, first=(rep == 0)):
                b()
            for jj in range(n_jj):
                # qkv chunk jj must be fully emitted before attn(*, jj)
                drain_chunk(jj)
                if jj + 1 < n_tch:
                    add_filler(jj + 1, qkv_chunk_bundles(jj + 1))
                if jj == n_jj - 1:
                    # out-proj of jj 0..2 feeds PE during the filler-starved
                    # final attention sweep
                    for j2 in range(n_jj - 1):
                        add_filler(-1, out_proj_bundles(j2))
                nxt = jj + 1 if jj + 1 < n_tch else None
                quota0 = pending_chunk.get(nxt, 0) if nxt is not None else 0
                for pair in range(NP):
                    emit_attn(pair, jj)
                    fill(3)
                    if nxt is not None:
                        quota = (NP - 1 - pair) * quota0 // NP
                        while pending_chunk.get(nxt, 0) > quota:
                            _pop()
            add_filler(-1, out_proj_bundles(n_jj - 1, tail=True))
            while filler:
                _pop()

    nc.compile()
    return nc


def host_consts(t=T):
    pos = np.arange(t, dtype=np.float32)[:, None]
    i = np.arange(0, D, 2, dtype=np.float32)[None, :]
    theta = pos / np.power(np.float32(10000.0), i / np.float32(D))
    cos = np.cos(theta).astype(np.float32)  # [t, 32]
    sin = np.sin(theta).astype(np.float32)
    cos2 = np.ascontiguousarray(np.tile(cos.T, (4, 1))).astype(NPBF)              # [128, t]
    sinS = np.ascontiguousarray(
        np.tile(np.concatenate([-sin.T, sin.T], 0), (2, 1))
    ).astype(NPBF)                                                                # [128, t]
    r = np.arange(P)[:, None]
    c = np.arange(P)[None, :]
    maskn = (r <= c).astype(NPBF)
    ident = np.eye(P).astype(NPBF)
    return cos2, sinS, maskn, ident


def make_in_maps(x, w_qkv, w_out):
    x = np.asarray(x, np.float32)
    w_qkv = np.asarray(w_qkv, np.float32).astype(NPBF)
    w_out = np.asarray(w_out, np.float32).astype(NPBF)
    cos2, sinS, maskn, ident = host_consts()
    in_maps = []
    for c0 in range(NCORES):
        b, g = c0 // 2, c0 % 2
        h0 = g * F
        xTb = np.ascontiguousarray(x[b].T.astype(NPBF))
        in_maps.append({
            "xT": xTb,
            "wq": np.ascontiguousarray(w_qkv[:, h0 : h0 + F]),
            "wk": np.ascontiguousarray(w_qkv[:, C + h0 : C + h0 + F]),
            "wv": np.ascontiguousarray(w_qkv[:, 2 * C + h0 : 2 * C + h0 + F]),
            "wo": np.ascontiguousarray(w_out[h0 : h0 + F, :]),
            "cos2": cos2, "sinS": sinS, "maskn": maskn, "ident": ident,
        })
    return in_maps


_REPL = {"cos2", "sinS", "maskn", "ident"}


class _Runner:
    """jit-once SPMD runner over jax.shard_map + the bass_exec custom call.

    Used instead of bass_utils.run_bass_kernel_spmd because the donation
    path in run_bass_via_pjrt hits NRT_EXEC_UNIT_UNRECOVERABLE at this
    problem size; passing non-donated zero output buffers (the kernel fully
    overwrites y) is stable. Replicating the shared inputs (rope/mask
    constants) also trims host->device traffic.
    """

    def __init__(self, nc, n_cores):
        import jax
        from jax.sharding import Mesh, PartitionSpec as PSpec
        from concourse import bass2jax

        bass2jax.install_neuronx_cc_hook()
        self.jax = jax
        self.n_cores = n_cores
        part_name = nc.partition_id_tensor.name if nc.partition_id_tensor else None
        in_names, out_names, out_avals, zero_outs = [], [], [], []
        for alloc in nc.m.functions[0].allocations:
            if not isinstance(alloc, mybir.MemoryLocationSet):
                continue
            name = alloc.memorylocations[0].name
            if alloc.kind == "ExternalInput":
                if name != part_name:
                    in_names.append(name)
            elif alloc.kind == "ExternalOutput":
                out_names.append(name)
                shape = tuple(alloc.tensor_shape)
                dtype = mybir.dt.np(alloc.dtype)
                out_avals.append(jax.core.ShapedArray(shape, dtype))
                zero_outs.append(np.zeros(shape, dtype))
        self.in_names, self.out_names = in_names, out_names
        self.out_avals, self.zero_outs = out_avals, zero_outs
        all_names = in_names + out_names + ([part_name] if part_name else [])

        def _body(*args):
            operands = list(args)
            if part_name is not None:
                operands.append(bass2jax.partition_id_tensor())
            outs = bass2jax._bass_exec_p.bind(
                *operands,
                out_avals=tuple(out_avals),
                in_names=tuple(all_names),
                out_names=tuple(out_names),
                lowering_input_output_aliases=(),
                sim_require_finite=False,
                sim_require_nnan=False,
                nc=nc,
            )
            return tuple(outs)

        try:
            from jax.experimental.shard_map import shard_map
        except ImportError:
            from jax.shard_map import shard_map
        devices = jax.devices()[:n_cores]
        self.mesh = Mesh(np.asarray(devices), ("core",))
        in_specs = tuple(
            PSpec() if nm in _REPL else PSpec("core") for nm in in_names
        ) + tuple(PSpec("core") for _ in out_names)
        out_specs = tuple(PSpec("core") for _ in out_names)
        self.fn = jax.jit(
            shard_map(_body, mesh=self.mesh, in_specs=in_specs,
                      out_specs=out_specs, check_rep=False),
            keep_unused=True,
        )

    def run(self, in_maps):
        args = []
        for nm in self.in_names:
            if nm in _REPL:
                args.append(np.asarray(in_maps[0][nm]))
            else:
                args.append(np.concatenate([np.asarray(m[nm]) for m in in_maps], axis=0))
        for z in self.zero_outs:
            args.append(np.zeros((self.n_cores * z.shape[0], *z.shape[1:]), z.dtype))
        outs = self.jax.block_until_ready(self.fn(*args))
        res = []
        for c in range(self.n_cores):
            res.append({
                nm: np.asarray(o).reshape(self.n_cores, *aval.shape)[c]
                for nm, aval, o in zip(self.out_names, self.out_avals, outs)
            })
        return res


_cache = {}


def kernel(x, w_qkv, w_out):
    if "runner" not in _cache:
        _cache["nc"] = build_nc()
        _cache["runner"] = _Runner(_cache["nc"], NCORES)
    in_maps = make_in_maps(x, w_qkv, w_out)
    results = _cache["runner"].run(in_maps)
    y = np.zeros((B, T, C), np.float32)
    for c0 in range(NCORES):
        b = c0 // 2
        y[b] += results[c0]["y"]
    return y


# revision 30
# speedup vs baseline: 1.0036x; 1.0036x over previous
import sys

sys.path.insert(0, "/opt/trn_rl_repo")

from collections import deque
from contextlib import ExitStack

import numpy as np
import ml_dtypes

import concourse.bass as bass
import concourse.tile as tile
from concourse import bacc
from concourse import mybir

B, T, C = 4, 2048, 1024
NH, D = 16, 64
NCORES = 8
# core c owns batch c//2 and head-group c%2 (8 heads = 4 pairs of 2)
NP = 4                      # head pairs per core
F = NP * 2 * D              # per-core feature slice (512)
P = 128
TCH = 512                   # token chunk for qkv phase
QCH = 512                   # q chunk in attention
KT = 128                    # k tile in attention
f32 = mybir.dt.float32
f32r = mybir.dt.float32r
bf16 = mybir.dt.bfloat16
AF = mybir.ActivationFunctionType
NPBF = ml_dtypes.bfloat16


def build_nc(reps=1):
    """One-core SPMD program: this core owns one batch and 8 heads (4 pairs)."""
    t = T
    n_cc = C // P               # contraction chunks for qkv (8)
    n_tch = t // TCH            # token chunks (4)
    n_jj = t // QCH             # q chunks (4)
    n_kt_b = t // KT            # k tiles (16)

    nc = bacc.Bacc(None, target_bir_lowering=False)
    xT = nc.declare_dram_parameter("xT", [C, t], bf16, isOutput=False)
    wq = nc.declare_dram_parameter("wq", [C, F], bf16, isOutput=False)
    wk = nc.declare_dram_parameter("wk", [C, F], bf16, isOutput=False)
    wv = nc.declare_dram_parameter("wv", [C, F], bf16, isOutput=False)
    wo = nc.declare_dram_parameter("wo", [F, C], bf16, isOutput=False)
    cos2 = nc.declare_dram_parameter("cos2", [P, t], bf16, isOutput=False)
    sinS = nc.declare_dram_parameter("sinS", [P, t], bf16, isOutput=False)
    maskn = nc.declare_dram_parameter("maskn", [P, P], bf16, isOutput=False)
    ident = nc.declare_dram_parameter("ident", [P, P], bf16, isOutput=False)
    y = nc.declare_dram_parameter("y", [t, C], f32, isOutput=True)

    xT_t = xT.rearrange("(o p) n -> p o n", p=P)   # [128, 8, t]

    with tile.TileContext(nc) as tc, ExitStack() as ctx:
        consts = ctx.enter_context(tc.tile_pool(name="consts", bufs=1))
        xpool = ctx.enter_context(tc.tile_pool(name="xt", bufs=3))
        rope = ctx.enter_context(tc.tile_pool(name="rope", bufs=3))
        ppool = ctx.enter_context(tc.tile_pool(name="pp", bufs=4))
        otp = ctx.enter_context(tc.tile_pool(name="otp", bufs=3))
        ystage = ctx.enter_context(tc.tile_pool(name="yst", bufs=3))
        ps_s = ctx.enter_context(tc.tile_pool(name="pss", bufs=2, space="PSUM"))
        ps_o = ctx.enter_context(tc.tile_pool(name="pso", bufs=1, space="PSUM"))
        ps_q = ctx.enter_context(tc.tile_pool(name="psq", bufs=2, space="PSUM"))

        # resident constants (wq first, split per-cc: the first matmul only
        # needs cc=0, so start computing after ~1/8 of the load)
        wq_r = wq.rearrange("(o p) f -> p o f", p=P)
        wq_sb = consts.tile([P, n_cc, F], bf16)
        for h in range(2):
            nc.sync.dma_start(wq_sb[:, 4 * h : 4 * h + 4], wq_r[:, 4 * h : 4 * h + 4])
        wk_r = wk.rearrange("(o p) f -> p o f", p=P)
        wk_sb = consts.tile([P, n_cc, F], bf16)
        for h in range(2):
            nc.scalar.dma_start(wk_sb[:, 4 * h : 4 * h + 4], wk_r[:, 4 * h : 4 * h + 4])
        wv_r = wv.rearrange("(o p) f -> p o f", p=P)
        wv_sb = consts.tile([P, n_cc, F], bf16)
        for h in range(2):
            nc.scalar.dma_start(wv_sb[:, 4 * h : 4 * h + 4], wv_r[:, 4 * h : 4 * h + 4])
        cos_sb = consts.tile([P, t], bf16)
        nc.scalar.dma_start(cos_sb, cos2[:, :])
        sin_sb = consts.tile([P, t], bf16)
        nc.scalar.dma_start(sin_sb, sinS[:, :])
        wo_sb = consts.tile([P, NP, C], bf16)
        nc.scalar.dma_start(wo_sb, wo.rearrange("(r p) c -> p r c", p=P))
        mask_sb = consts.tile([P, P], bf16)
        nc.scalar.dma_start(mask_sb, maskn[:, :])
        id_sb = consts.tile([P, P], bf16)
        nc.scalar.dma_start(id_sb, ident[:, :])

        # persistent per-pair tensors
        q_sb = consts.tile([P, NP, t], bf16, name="q_sb")
        k_sb = consts.tile([P, NP, t], bf16, name="k_sb")
        v1_sb = consts.tile([P, NP, n_kt_b, 2, D + 1], bf16, name="v1_sb")
        nc.vector.memset(v1_sb[:, :, :, :, D], 1.0)
        # attention output staging (all jj live): [feat, jj, pair, tok]
        ot_sb = consts.tile([P, n_jj, NP, QCH], bf16, name="ot_sb")

        H = D // 2  # 32

        def rope_apply(ps, dst, c0, bulk_pool=False):
            """dst(bf16) = ps*cos + swap32(ps)*sin, feature-major [128, TCH].

            psum reads must be DVE/ACT (GPSIMD cannot access PSUM); the final
            all-SBUF add goes to Pool to keep DVE lean."""
            cos_sl = cos_sb[:, c0 : c0 + TCH]
            sin_sl = sin_sb[:, c0 : c0 + TCH]
            raw = rope.tile([P, TCH], bf16, tag="raw")
            nc.vector.tensor_copy(raw, ps)
            rot = rope.tile([P, TCH], bf16, tag="rot")
            for hh in range(2):
                b0 = hh * D
                nc.vector.tensor_copy(rot[b0 : b0 + H], raw[b0 + H : b0 + 2 * H])
                nc.vector.tensor_copy(rot[b0 + H : b0 + 2 * H], raw[b0 : b0 + H])
            mul_eng = nc.gpsimd if bulk_pool else nc.vector
            t1 = rope.tile([P, TCH], bf16, tag="t1")
            mul_eng.tensor_mul(out=t1, in0=raw, in1=cos_sl)
            t2 = rope.tile([P, TCH], bf16, tag="t2")
            mul_eng.tensor_mul(out=t2, in0=rot, in1=sin_sl)
            nc.gpsimd.tensor_add(out=dst, in0=t1, in1=t2)

        def qkv_chunk_bundles(it, first=False):
            """Emit-closures computing q/k/v for all pairs for token chunk it.

            All tiles are allocated lazily inside the closures so pool ring
            order matches actual emission order (bundles may be deferred)."""
            c0 = it * TCH
            bundles = []
            cell = {}
            def load(cc):
                def go():
                    if "xt" not in cell:
                        cell["xt"] = xpool.tile([P, n_cc, TCH], bf16, tag="xt", name="xt")
                    eng = nc.gpsimd if first else nc.sync
                    eng.dma_start(cell["xt"][:, cc], xT_t[:, cc, c0 : c0 + TCH])
                return go

            def load_half(h):
                # chunk 0 only: one DMA per half - fewer DGE generations at
                # the cold start
                def go():
                    if "xt" not in cell:
                        cell["xt"] = xpool.tile([P, n_cc, TCH], bf16, tag="xt", name="xt")
                    nc.gpsimd.dma_start(
                        cell["xt"][:, 4 * h : 4 * h + 4],
                        xT_t[:, 4 * h : 4 * h + 4, c0 : c0 + TCH],
                    )
                return go

            if first:
                bundles.append(load_half(0))
                bundles.append(load_half(1))
            else:
                for cc in range(n_cc):
                    bundles.append(load(cc))

            def alloc_ps(shape, key):
                # chunk 0 runs before attention: cycle its psum tiles through
                # the (idle) score pool too, doubling rotation depth
                if first:
                    n = cell.get("_pc", 0)
                    cell["_pc"] = n + 1
                    pool, tg = [(ps_q, "pq"), (ps_s, "mm")][n % 2]
                else:
                    pool, tg = ps_q, "pq"
                cell[key] = pool.tile(shape, f32, tag=tg, name=key)

            def qk_mms(w_sb, pair, key, lohi, alloc=False):
                def go():
                    if alloc:
                        alloc_ps([P, TCH], key)
                    ps = cell[key]
                    for cc in lohi:
                        nc.tensor.matmul(
                            ps,
                            lhsT=w_sb[:, cc, pair * P : (pair + 1) * P],
                            rhs=cell["xt"][:, cc, :],
                            start=(cc == 0),
                            stop=(cc == n_cc - 1),
                        )
                return go

            def rope_fin(key, dst_sb, pair):
                return lambda: rope_apply(cell[key], dst_sb[:, pair, c0 : c0 + TCH],
                                          c0, bulk_pool=first)

            def v_tile(pair, s):
                def go():
                    key = f"psv{pair}"
                    if s == 0:
                        alloc_ps([P, TCH // P, 2, D], key)
                    ps = cell[key]
                    for cc in range(n_cc):
                        nc.tensor.matmul(
                            ps[:, s],
                            lhsT=cell["xt"][:, cc, s * P : (s + 1) * P],
                            rhs=wv_sb[:, cc, pair * P : (pair + 1) * P],
                            start=(cc == 0),
                            stop=(cc == n_cc - 1),
                        )
                    ti = it * (TCH // P) + s
                    nc.vector.tensor_copy(v1_sb[:, pair, ti, :, 0:D], ps[:, s])
                return go

            for pair in range(NP):
                kq, kk = f"psq{pair}", f"psk{pair}"
                bundles.append(qk_mms(wq_sb, pair, kq, range(0, 4), alloc=True))
                bundles.append(qk_mms(wq_sb, pair, kq, range(4, 8)))
                bundles.append(rope_fin(kq, q_sb, pair))
                bundles.append(qk_mms(wk_sb, pair, kk, range(0, 4), alloc=True))
                bundles.append(qk_mms(wk_sb, pair, kk, range(4, 8)))
                bundles.append(rope_fin(kk, k_sb, pair))
                for s in range(TCH // P):
                    bundles.append(v_tile(pair, s))
            return bundles

        def out_proj_bundles(jj, tail=False):
            """Emit-closures for the out-projection of q-chunk jj (all pairs)."""
            bundles = []

            def psy_tile(qt, nh, n):
                def go():
                    if tail:
                        # attention is over: borrow the idle score pool for a
                        # wider psY rotation, and put ysb copies on both DVE
                        # and the now-idle ACT
                        pool, tg = [(ps_q, "pq"), (ps_s, "mm")][n % 2]
                        psY = pool.tile([P, 512], f32, tag=tg, name="psY")
                    else:
                        psY = ps_q.tile([P, 512], f32, tag="pq", name="psY")
                    for pair in range(NP):
                        nc.tensor.matmul(
                            psY,
                            lhsT=ot_sb[:, jj, pair, qt * P : (qt + 1) * P],
                            rhs=wo_sb[:, pair, nh * 512 : (nh + 1) * 512],
                            start=(pair == 0),
                            stop=(pair == NP - 1),
                        )
                    ysb = ystage.tile([P, 512], f32, tag="ysb")
                    if tail and n % 2 == 1:
                        nc.scalar.copy(ysb, psY)
                    else:
                        nc.vector.tensor_copy(ysb, psY)
                    r0 = jj * QCH + qt * P
                    if tail:
                        deng = nc.sync if n % 2 == 0 else nc.gpsimd
                    else:
                        deng = nc.sync
                    deng.dma_start(y[r0 : r0 + P, nh * 512 : (nh + 1) * 512], ysb)
                return go

            n = 0
            for qt in range(QCH // P):
                for nh in range(C // 512):
                    bundles.append(psy_tile(qt, nh, n))
                    n += 1
            return bundles

        filler = deque()          # entries: (chunk_idx_or_-1, closure)
        pending_chunk = {}
        # total fill() slots in the attention phase, for supply pacing:
        # per (jj, pair): (n_kt-1)*2 inside + 3 after the block
        pace = {"slots": sum(4 * ((4 * jj + 4 - 1) * 2 + 3) for jj in range(n_jj)),
                "acc": 0.0}

        def _pop():
            tag, fn = filler.popleft()
            if tag >= 0:
                pending_chunk[tag] -= 1
            fn()

        def fill(k):
            for _ in range(k):
                pace["slots"] -= 1
                if not filler:
                    continue
                # spread the available bundles over the remaining slots
                pace["acc"] += min(4.0, len(filler) / max(1, pace["slots"]))
                while filler and pace["acc"] >= 1.0:
                    pace["acc"] -= 1.0
                    _pop()

        def add_filler(tag, bundles):
            if tag >= 0:
                pending_chunk[tag] = pending_chunk.get(tag, 0) + len(bundles)
            filler.extend((tag, b) for b in bundles)

        def drain_chunk(cidx):
            while pending_chunk.get(cidx, 0) > 0:
                _pop()

        def emit_attn(pair, jj):
            n_kt = 4 * jj + 4
            psO = ps_o.tile([P, 2, QCH], f32, tag="o0")
            tiles = []

            def score(i):
                lo = max(0, i * KT - jj * QCH)
                qs = jj * QCH + lo
                psS = ps_s.tile([P, 2, QCH], f32, tag="mm")
                kt_sl = slice(i * KT, (i + 1) * KT)
                q_sl = slice(qs, (jj + 1) * QCH)
                for h in range(2):
                    nc.tensor.matmul(
                        psS[:, h, lo:],
                        lhsT=k_sb[h * D : (h + 1) * D, pair, kt_sl],
                        rhs=q_sb[h * D : (h + 1) * D, pair, q_sl],
                        start=True,
                        stop=True,
                    )
                Pp = ppool.tile([P, 2, QCH], bf16, tag="p0")
                nc.scalar.activation(Pp[:, :, lo:], psS[:, :, lo:], AF.Exp, scale=0.125)
                if i >= 4 * jj:  # diagonal tile: triangle mask both heads
                    nc.vector.tensor_mul(
                        Pp[:, :, lo : lo + P],
                        Pp[:, :, lo : lo + P],
                        mask_sb[:, None, :].to_broadcast((P, 2, P)),
                    )
                return lo, Pp

            def pv(i, lo, Pp):
                for h in range(2):
                    nc.tensor.matmul(
                        psO[0 : D + 1, h, lo:],
                        lhsT=v1_sb[:, pair, i, h, :],
                        rhs=Pp[:, h, lo:],
                        start=(i == 0),
                        stop=(i == n_kt - 1),
                    )

            for i in range(n_kt):
                tiles.append(score(i))
                if i > 0:
                    # filler goes between S(i) and PV(i-1): that's where the
                    # in-order PE queue would otherwise wait on exp(i-1)
                    fill(2)
                    pv(i - 1, *tiles[i - 1])
            pv(n_kt - 1, *tiles[n_kt - 1])

            # Copy psO out first (DVE: GPSIMD cannot read PSUM): frees the
            # psum bank for the next (pair, jj) block earlier than the
            # recip/mul chain would.
            oc = otp.tile([D + 1, 2, QCH], f32, tag="oc")
            nc.vector.tensor_copy(oc, psO[0 : D + 1, :, :])
            # softmax normalization: OT[h] = O[h] / l[h]; muls on Pool
            rs = otp.tile([1, 2, QCH], f32, tag="rs")
            nc.vector.reciprocal(rs, oc[D : D + 1, :, :])
            rb0 = otp.tile([D, QCH], f32, tag="rb0")
            nc.gpsimd.partition_broadcast(rb0, rs[0:1, 0, :])
            rb1 = otp.tile([D, QCH], f32, tag="rb1")
            nc.gpsimd.partition_broadcast(rb1, rs[0:1, 1, :])
            if jj == n_jj - 1 and pair == NP - 1:
                for qt in range(QCH // P):
                    qsl = slice(qt * P, (qt + 1) * P)
                    nc.vector.tensor_mul(ot_sb[0:D, jj, pair, qsl], oc[0:D, 0, qsl], rb0[:, qsl])
                    nc.vector.tensor_mul(ot_sb[D:P, jj, pair, qsl], oc[0:D, 1, qsl], rb1[:, qsl])
            else:
                nc.gpsimd.tensor_mul(ot_sb[0:D, jj, pair, :], oc[0:D, 0, :], rb0)
                nc.gpsimd.tensor_mul(ot_sb[D:P, jj, pair, :], oc[0:D, 1, :], rb1)

        # ---- emission schedule ----
        for rep in range(reps):
            # chunk 0 emitted directly; chunk jj+1 rides as filler inside
            # attn(*, jj) so attention (and ACT exp work) starts early
            for b in qkv_chunk_bundles(0, first=(rep == 0)):
                b()
            for jj in range(n_jj):
                # qkv chunk jj must be fully emitted before attn(*, jj)
                drain_chunk(jj)
                if jj + 1 < n_tch:
                    add_filler(jj + 1, qkv_chunk_bundles(jj + 1))
                if jj == n_jj - 1:
                    # out-proj of jj 0..2 feeds PE during the filler-starved
                    # final attention sweep
                    for j2 in range(n_jj - 1):
                        add_filler(-1, out_proj_bundles(j2))
                nxt = jj + 1 if jj + 1 < n_tch else None
                quota0 = pending_chunk.get(nxt, 0) if nxt is not None else 0
                for pair in range(NP):
                    emit_attn(pair, jj)
                    fill(3)
                    if nxt is not None:
                        quota = (NP - 1 - pair) * quota0 // NP
                        while pending_chunk.get(nxt, 0) > quota:
                            _pop()
            add_filler(-1, out_proj_bundles(n_jj - 1, tail=True))
            while filler:
                _pop()

    nc.compile()
    return nc


def host_consts(t=T):
    pos = np.arange(t, dtype=np.float32)[:, None]
    i = np.arange(0, D, 2, dtype=np.float32)[None, :]
    theta = pos / np.power(np.float32(10000.0), i / np.float32(D))
    cos = np.cos(theta).astype(np.float32)  # [t, 32]
    sin = np.sin(theta).astype(np.float32)
    cos2 = np.ascontiguousarray(np.tile(cos.T, (4, 1))).astype(NPBF)              # [128, t]
    sinS = np.ascontiguousarray(
        np.tile(np.concatenate([-sin.T, sin.T], 0), (2, 1))
    ).astype(NPBF)                                                                # [128, t]
    r = np.arange(P)[:, None]
    c = np.arange(P)[None, :]
    maskn = (r <= c).astype(NPBF)
    ident = np.eye(P).astype(NPBF)
    return cos2, sinS, maskn, ident


def make_in_maps(x, w_qkv, w_out):
    x = np.asarray(x, np.float32)
    w_qkv = np.asarray(w_qkv, np.float32).astype(NPBF)
    w_out = np.asarray(w_out, np.float32).astype(NPBF)
    cos2, sinS, maskn, ident = host_consts()
    in_maps = []
    for c0 in range(NCORES):
        b, g = c0 // 2, c0 % 2
        h0 = g * F
        xTb = np.ascontiguousarray(x[b].T.astype(NPBF))
        in_maps.append({
            "xT": xTb,
            "wq": np.ascontiguousarray(w_qkv[:, h0 : h0 + F]),
            "wk": np.ascontiguousarray(w_qkv[:, C + h0 : C + h0 + F]),
            "wv": np.ascontiguousarray(w_qkv[:, 2 * C + h0 : 2 * C + h0 + F]),
            "wo": np.ascontiguousarray(w_out[h0 : h0 + F, :]),
            "cos2": cos2, "sinS": sinS, "maskn": maskn, "ident": ident,
        })
    return in_maps


_REPL = {"cos2", "sinS", "maskn", "ident"}


class _Runner:
    """jit-once SPMD runner over jax.shard_map + the bass_exec custom call.

    Used instead of bass_utils.run_bass_kernel_spmd because the donation
    path in run_bass_via_pjrt hits NRT_EXEC_UNIT_UNRECOVERABLE at this
    problem size; passing non-donated zero output buffers (the kernel fully
    overwrites y) is stable. Replicating the shared inputs (rope/mask
    constants) also trims host->device traffic.
    """

    def __init__(self, nc, n_cores):
        import jax
        from jax.sharding import Mesh, PartitionSpec as PSpec
        from concourse import bass2jax

        bass2jax.install_neuronx_cc_hook()
        self.jax = jax
        self.n_cores = n_cores
        part_name = nc.partition_id_tensor.name if nc.partition_id_tensor else None
        in_names, out_names, out_avals, zero_outs = [], [], [], []
        for alloc in nc.m.functions[0].allocations:
            if not isinstance(alloc, mybir.MemoryLocationSet):
                continue
            name = alloc.memorylocations[0].name
            if alloc.kind == "ExternalInput":
                if name != part_name:
                    in_names.append(name)
            elif alloc.kind == "ExternalOutput":
                out_names.append(name)
                shape = tuple(alloc.tensor_shape)
                dtype = mybir.dt.np(alloc.dtype)
                out_avals.append(jax.core.ShapedArray(shape, dtype))
                zero_outs.append(np.zeros(shape, dtype))
        self.in_names, self.out_names = in_names, out_names
        self.out_avals, self.zero_outs = out_avals, zero_outs
        all_names = in_names + out_names + ([part_name] if part_name else [])

        def _body(*args):
            operands = list(args)
            if part_name is not None:
                operands.append(bass2jax.partition_id_tensor())
            outs = bass2jax._bass_exec_p.bind(
                *operands,
                out_avals=tuple(out_avals),
                in_names=tuple(all_names),
                out_names=tuple(out_names),
                lowering_input_output_aliases=(),
                sim_require_finite=False,
                sim_require_nnan=False,
                nc=nc,
            )
            return tuple(outs)

        try:
            from jax.experimental.shard_map import shard_map
        except ImportError:
            from jax.shard_map import shard_map
        devices = jax.devices()[:n_cores]
        self.mesh = Mesh(np.asarray(devices), ("core",))
        in_specs = tuple(
            PSpec() if nm in _REPL else PSpec("core") for nm in in_names
        ) + tuple(PSpec("core") for _ in out_names)
        out_specs = tuple(PSpec("core") for _ in out_names)
        self.fn = jax.jit(
            shard_map(_body, mesh=self.mesh, in_specs=in_specs,
                      out_specs=out_specs, check_rep=False),
            keep_unused=True,
        )

    def run(self, in_maps):
        args = []
        for nm in self.in_names:
            if nm in _REPL:
                args.append(np.asarray(in_maps[0][nm]))
            else:
                args.append(np.concatenate([np.asarray(m[nm]) for m in in_maps], axis=0))
        for z in self.zero_outs:
            args.append(np.zeros((self.n_cores * z.shape[0], *z.shape[1:]), z.dtype))
        outs = self.jax.block_until_ready(self.fn(*args))
        res = []
        for c in range(self.n_cores):
            res.append({
                nm: np.asarray(o).reshape(self.n_cores, *aval.shape)[c]
                for nm, aval, o in zip(self.out_names, self.out_avals, outs)
            })
        return res


_cache = {}


def kernel(x, w_qkv, w_out):
    if "runner" not in _cache:
        _cache["nc"] = build_nc()
        _cache["runner"] = _Runner(_cache["nc"], NCORES)
    in_maps = make_in_maps(x, w_qkv, w_out)
    results = _cache["runner"].run(in_maps)
    y = np.zeros((B, T, C), np.float32)
    for c0 in range(NCORES):
        b = c0 // 2
        y[b] += results[c0]["y"]
    return y


# revision 31
# speedup vs baseline: 1.0088x; 1.0052x over previous
import sys

sys.path.insert(0, "/opt/trn_rl_repo")

from collections import deque
from contextlib import ExitStack

import numpy as np
import ml_dtypes

import concourse.bass as bass
import concourse.tile as tile
from concourse import bacc
from concourse import mybir

B, T, C = 4, 2048, 1024
NH, D = 16, 64
NCORES = 8
# core c owns batch c//2 and head-group c%2 (8 heads = 4 pairs of 2)
NP = 4                      # head pairs per core
F = NP * 2 * D              # per-core feature slice (512)
P = 128
TCH = 512                   # token chunk for qkv phase
QCH = 512                   # q chunk in attention
KT = 128                    # k tile in attention
f32 = mybir.dt.float32
f32r = mybir.dt.float32r
bf16 = mybir.dt.bfloat16
AF = mybir.ActivationFunctionType
NPBF = ml_dtypes.bfloat16


def build_nc(reps=1):
    """One-core SPMD program: this core owns one batch and 8 heads (4 pairs)."""
    t = T
    n_cc = C // P               # contraction chunks for qkv (8)
    n_tch = t // TCH            # token chunks (4)
    n_jj = t // QCH             # q chunks (4)
    n_kt_b = t // KT            # k tiles (16)

    nc = bacc.Bacc(None, target_bir_lowering=False)
    xT = nc.declare_dram_parameter("xT", [C, t], bf16, isOutput=False)
    wq = nc.declare_dram_parameter("wq", [C, F], bf16, isOutput=False)
    wk = nc.declare_dram_parameter("wk", [C, F], bf16, isOutput=False)
    wv = nc.declare_dram_parameter("wv", [C, F], bf16, isOutput=False)
    wo = nc.declare_dram_parameter("wo", [F, C], bf16, isOutput=False)
    cos2 = nc.declare_dram_parameter("cos2", [P, t], bf16, isOutput=False)
    sinS = nc.declare_dram_parameter("sinS", [P, t], bf16, isOutput=False)
    maskn = nc.declare_dram_parameter("maskn", [P, P], bf16, isOutput=False)
    ident = nc.declare_dram_parameter("ident", [P, P], bf16, isOutput=False)
    y = nc.declare_dram_parameter("y", [t, C], f32, isOutput=True)

    xT_t = xT.rearrange("(o p) n -> p o n", p=P)   # [128, 8, t]

    with tile.TileContext(nc) as tc, ExitStack() as ctx:
        consts = ctx.enter_context(tc.tile_pool(name="consts", bufs=1))
        xpool = ctx.enter_context(tc.tile_pool(name="xt", bufs=3))
        rope = ctx.enter_context(tc.tile_pool(name="rope", bufs=3))
        ppool = ctx.enter_context(tc.tile_pool(name="pp", bufs=4))
        otp = ctx.enter_context(tc.tile_pool(name="otp", bufs=3))
        ystage = ctx.enter_context(tc.tile_pool(name="yst", bufs=3))
        ps_s = ctx.enter_context(tc.tile_pool(name="pss", bufs=2, space="PSUM"))
        ps_o = ctx.enter_context(tc.tile_pool(name="pso", bufs=1, space="PSUM"))
        ps_q = ctx.enter_context(tc.tile_pool(name="psq", bufs=2, space="PSUM"))

        # resident constants (wq first, split per-cc: the first matmul only
        # needs cc=0, so start computing after ~1/8 of the load)
        wq_r = wq.rearrange("(o p) f -> p o f", p=P)
        wq_sb = consts.tile([P, n_cc, F], bf16)
        for h in range(2):
            nc.sync.dma_start(wq_sb[:, 4 * h : 4 * h + 4], wq_r[:, 4 * h : 4 * h + 4])
        wk_r = wk.rearrange("(o p) f -> p o f", p=P)
        wk_sb = consts.tile([P, n_cc, F], bf16)
        for h in range(2):
            nc.scalar.dma_start(wk_sb[:, 4 * h : 4 * h + 4], wk_r[:, 4 * h : 4 * h + 4])
        wv_r = wv.rearrange("(o p) f -> p o f", p=P)
        wv_sb = consts.tile([P, n_cc, F], bf16)
        for h in range(2):
            nc.scalar.dma_start(wv_sb[:, 4 * h : 4 * h + 4], wv_r[:, 4 * h : 4 * h + 4])
        cos_sb = consts.tile([P, t], bf16)
        nc.scalar.dma_start(cos_sb, cos2[:, :])
        sin_sb = consts.tile([P, t], bf16)
        nc.scalar.dma_start(sin_sb, sinS[:, :])
        wo_sb = consts.tile([P, NP, C], bf16)
        nc.scalar.dma_start(wo_sb, wo.rearrange("(r p) c -> p r c", p=P))
        mask_sb = consts.tile([P, P], bf16)
        nc.scalar.dma_start(mask_sb, maskn[:, :])
        id_sb = consts.tile([P, P], bf16)
        nc.scalar.dma_start(id_sb, ident[:, :])

        # persistent per-pair tensors
        q_sb = consts.tile([P, NP, t], bf16, name="q_sb")
        k_sb = consts.tile([P, NP, t], bf16, name="k_sb")
        v1_sb = consts.tile([P, NP, n_kt_b, 2, D + 1], bf16, name="v1_sb")
        nc.vector.memset(v1_sb[:, :, :, :, D], 1.0)
        # attention output staging (all jj live): [feat, jj, pair, tok]
        ot_sb = consts.tile([P, n_jj, NP, QCH], bf16, name="ot_sb")

        H = D // 2  # 32

        def rope_apply(ps, dst, c0, bulk_pool=False):
            """dst(bf16) = ps*cos + swap32(ps)*sin, feature-major [128, TCH].

            psum reads must be DVE/ACT (GPSIMD cannot access PSUM); the final
            all-SBUF add goes to Pool to keep DVE lean."""
            cos_sl = cos_sb[:, c0 : c0 + TCH]
            sin_sl = sin_sb[:, c0 : c0 + TCH]
            raw = rope.tile([P, TCH], bf16, tag="raw")
            nc.vector.tensor_copy(raw, ps)
            rot = rope.tile([P, TCH], bf16, tag="rot")
            for hh in range(2):
                b0 = hh * D
                nc.vector.tensor_copy(rot[b0 : b0 + H], raw[b0 + H : b0 + 2 * H])
                nc.vector.tensor_copy(rot[b0 + H : b0 + 2 * H], raw[b0 : b0 + H])
            mul_eng = nc.gpsimd if bulk_pool else nc.vector
            t1 = rope.tile([P, TCH], bf16, tag="t1")
            mul_eng.tensor_mul(out=t1, in0=raw, in1=cos_sl)
            t2 = rope.tile([P, TCH], bf16, tag="t2")
            mul_eng.tensor_mul(out=t2, in0=rot, in1=sin_sl)
            nc.gpsimd.tensor_add(out=dst, in0=t1, in1=t2)

        def qkv_chunk_bundles(it, first=False):
            """Emit-closures computing q/k/v for all pairs for token chunk it.

            All tiles are allocated lazily inside the closures so pool ring
            order matches actual emission order (bundles may be deferred)."""
            c0 = it * TCH
            bundles = []
            cell = {}
            def load(cc):
                def go():
                    if "xt" not in cell:
                        cell["xt"] = xpool.tile([P, n_cc, TCH], bf16, tag="xt", name="xt")
                    eng = nc.gpsimd if first else nc.sync
                    eng.dma_start(cell["xt"][:, cc], xT_t[:, cc, c0 : c0 + TCH])
                return go

            def load_half(h):
                # chunk 0 only: one DMA per half - fewer DGE generations at
                # the cold start
                def go():
                    if "xt" not in cell:
                        cell["xt"] = xpool.tile([P, n_cc, TCH], bf16, tag="xt", name="xt")
                    nc.gpsimd.dma_start(
                        cell["xt"][:, 4 * h : 4 * h + 4],
                        xT_t[:, 4 * h : 4 * h + 4, c0 : c0 + TCH],
                    )
                return go

            if first:
                bundles.append(load_half(0))
                bundles.append(load_half(1))
            else:
                for cc in range(n_cc):
                    bundles.append(load(cc))

            def alloc_ps(shape, key):
                # chunk 0 runs before attention: cycle its psum tiles through
                # the (idle) score pool too, doubling rotation depth
                if first:
                    n = cell.get("_pc", 0)
                    cell["_pc"] = n + 1
                    pool, tg = [(ps_q, "pq"), (ps_s, "mm")][n % 2]
                else:
                    pool, tg = ps_q, "pq"
                cell[key] = pool.tile(shape, f32, tag=tg, name=key)

            def qk_mms(w_sb, pair, key, lohi, alloc=False):
                def go():
                    if alloc:
                        alloc_ps([P, TCH], key)
                    ps = cell[key]
                    for cc in lohi:
                        nc.tensor.matmul(
                            ps,
                            lhsT=w_sb[:, cc, pair * P : (pair + 1) * P],
                            rhs=cell["xt"][:, cc, :],
                            start=(cc == 0),
                            stop=(cc == n_cc - 1),
                        )
                return go

            def rope_fin(key, dst_sb, pair):
                return lambda: rope_apply(cell[key], dst_sb[:, pair, c0 : c0 + TCH],
                                          c0, bulk_pool=first)

            def v_tile(pair, s):
                def go():
                    key = f"psv{pair}"
                    if s == 0:
                        alloc_ps([P, TCH // P, 2, D], key)
                    ps = cell[key]
                    for cc in range(n_cc):
                        nc.tensor.matmul(
                            ps[:, s],
                            lhsT=cell["xt"][:, cc, s * P : (s + 1) * P],
                            rhs=wv_sb[:, cc, pair * P : (pair + 1) * P],
                            start=(cc == 0),
                            stop=(cc == n_cc - 1),
                        )
                    ti = it * (TCH // P) + s
                    nc.vector.tensor_copy(v1_sb[:, pair, ti, :, 0:D], ps[:, s])
                return go

            for pair in range(NP):
                kq, kk = f"psq{pair}", f"psk{pair}"
                bundles.append(qk_mms(wq_sb, pair, kq, range(0, 4), alloc=True))
                bundles.append(qk_mms(wq_sb, pair, kq, range(4, 8)))
                bundles.append(rope_fin(kq, q_sb, pair))
                bundles.append(qk_mms(wk_sb, pair, kk, range(0, 4), alloc=True))
                bundles.append(qk_mms(wk_sb, pair, kk, range(4, 8)))
                bundles.append(rope_fin(kk, k_sb, pair))
                for s in range(TCH // P):
                    bundles.append(v_tile(pair, s))
            return bundles

        def out_proj_bundles(jj, tail=False):
            """Emit-closures for the out-projection of q-chunk jj (all pairs)."""
            bundles = []

            def psy_tile(qt, nh, n):
                def go():
                    if tail:
                        # attention is over: borrow the idle score pool for a
                        # wider psY rotation, and put ysb copies on both DVE
                        # and the now-idle ACT
                        pool, tg = [(ps_q, "pq"), (ps_s, "mm")][n % 2]
                        psY = pool.tile([P, 512], f32, tag=tg, name="psY")
                    else:
                        psY = ps_q.tile([P, 512], f32, tag="pq", name="psY")
                    for pair in range(NP):
                        nc.tensor.matmul(
                            psY,
                            lhsT=ot_sb[:, jj, pair, qt * P : (qt + 1) * P],
                            rhs=wo_sb[:, pair, nh * 512 : (nh + 1) * 512],
                            start=(pair == 0),
                            stop=(pair == NP - 1),
                        )
                    ysb = ystage.tile([P, 512], f32, tag="ysb")
                    if tail and n % 2 == 1:
                        nc.scalar.copy(ysb, psY)
                    else:
                        nc.vector.tensor_copy(ysb, psY)
                    r0 = jj * QCH + qt * P
                    nc.sync.dma_start(y[r0 : r0 + P, nh * 512 : (nh + 1) * 512], ysb)
                return go

            n = 0
            for qt in range(QCH // P):
                for nh in range(C // 512):
                    bundles.append(psy_tile(qt, nh, n))
                    n += 1
            return bundles

        filler = deque()          # entries: (chunk_idx_or_-1, closure)
        pending_chunk = {}
        # total fill() slots in the attention phase, for supply pacing:
        # per (jj, pair): (n_kt-1)*2 inside + 3 after the block
        pace = {"slots": sum(4 * ((4 * jj + 4 - 1) * 2 + 3) for jj in range(n_jj)),
                "acc": 0.0}

        def _pop():
            tag, fn = filler.popleft()
            if tag >= 0:
                pending_chunk[tag] -= 1
            fn()

        def fill(k):
            for _ in range(k):
                pace["slots"] -= 1
                if not filler:
                    continue
                # spread the available bundles over the remaining slots
                pace["acc"] += min(4.0, len(filler) / max(1, pace["slots"]))
                while filler and pace["acc"] >= 1.0:
                    pace["acc"] -= 1.0
                    _pop()

        def add_filler(tag, bundles):
            if tag >= 0:
                pending_chunk[tag] = pending_chunk.get(tag, 0) + len(bundles)
            filler.extend((tag, b) for b in bundles)

        def drain_chunk(cidx):
            while pending_chunk.get(cidx, 0) > 0:
                _pop()

        def emit_attn(pair, jj):
            n_kt = 4 * jj + 4
            psO = ps_o.tile([P, 2, QCH], f32, tag="o0")
            tiles = []

            def score(i):
                lo = max(0, i * KT - jj * QCH)
                qs = jj * QCH + lo
                psS = ps_s.tile([P, 2, QCH], f32, tag="mm")
                kt_sl = slice(i * KT, (i + 1) * KT)
                q_sl = slice(qs, (jj + 1) * QCH)
                for h in range(2):
                    nc.tensor.matmul(
                        psS[:, h, lo:],
                        lhsT=k_sb[h * D : (h + 1) * D, pair, kt_sl],
                        rhs=q_sb[h * D : (h + 1) * D, pair, q_sl],
                        start=True,
                        stop=True,
                    )
                Pp = ppool.tile([P, 2, QCH], bf16, tag="p0")
                nc.scalar.activation(Pp[:, :, lo:], psS[:, :, lo:], AF.Exp, scale=0.125)
                if i >= 4 * jj:  # diagonal tile: triangle mask both heads
                    nc.vector.tensor_mul(
                        Pp[:, :, lo : lo + P],
                        Pp[:, :, lo : lo + P],
                        mask_sb[:, None, :].to_broadcast((P, 2, P)),
                    )
                return lo, Pp

            def pv(i, lo, Pp):
                for h in range(2):
                    nc.tensor.matmul(
                        psO[0 : D + 1, h, lo:],
                        lhsT=v1_sb[:, pair, i, h, :],
                        rhs=Pp[:, h, lo:],
                        start=(i == 0),
                        stop=(i == n_kt - 1),
                    )

            for i in range(n_kt):
                tiles.append(score(i))
                if i > 0:
                    # filler goes between S(i) and PV(i-1): that's where the
                    # in-order PE queue would otherwise wait on exp(i-1)
                    fill(2)
                    pv(i - 1, *tiles[i - 1])
            pv(n_kt - 1, *tiles[n_kt - 1])

            # Copy psO out first (DVE: GPSIMD cannot read PSUM): frees the
            # psum bank for the next (pair, jj) block earlier than the
            # recip/mul chain would.
            oc = otp.tile([D + 1, 2, QCH], f32, tag="oc")
            nc.vector.tensor_copy(oc, psO[0 : D + 1, :, :])
            # softmax normalization: OT[h] = O[h] / l[h]; muls on Pool
            rs = otp.tile([1, 2, QCH], f32, tag="rs")
            nc.vector.reciprocal(rs, oc[D : D + 1, :, :])
            rb0 = otp.tile([D, QCH], f32, tag="rb0")
            nc.gpsimd.partition_broadcast(rb0, rs[0:1, 0, :])
            rb1 = otp.tile([D, QCH], f32, tag="rb1")
            nc.gpsimd.partition_broadcast(rb1, rs[0:1, 1, :])
            if jj == n_jj - 1 and pair == NP - 1:
                for qt in range(QCH // P):
                    qsl = slice(qt * P, (qt + 1) * P)
                    nc.vector.tensor_mul(ot_sb[0:D, jj, pair, qsl], oc[0:D, 0, qsl], rb0[:, qsl])
                    nc.vector.tensor_mul(ot_sb[D:P, jj, pair, qsl], oc[0:D, 1, qsl], rb1[:, qsl])
            else:
                nc.gpsimd.tensor_mul(ot_sb[0:D, jj, pair, :], oc[0:D, 0, :], rb0)
                nc.gpsimd.tensor_mul(ot_sb[D:P, jj, pair, :], oc[0:D, 1, :], rb1)

        # ---- emission schedule ----
        for rep in range(reps):
            # chunk 0 emitted directly; chunk jj+1 rides as filler inside
            # attn(*, jj) so attention (and ACT exp work) starts early
            for b in qkv_chunk_bundles(0, first=(rep == 0)):
                b()
            for jj in range(n_jj):
                # qkv chunk jj must be fully emitted before attn(*, jj)
                drain_chunk(jj)
                if jj + 1 < n_tch:
                    add_filler(jj + 1, qkv_chunk_bundles(jj + 1))
                if jj == n_jj - 1:
                    # out-proj of jj 0..2 feeds PE during the filler-starved
                    # final attention sweep
                    for j2 in range(n_jj - 1):
                        add_filler(-1, out_proj_bundles(j2))
                nxt = jj + 1 if jj + 1 < n_tch else None
                quota0 = pending_chunk.get(nxt, 0) if nxt is not None else 0
                for pair in range(NP):
                    emit_attn(pair, jj)
                    fill(3)
                    if nxt is not None:
                        quota = (NP - 1 - pair) * quota0 // NP
                        while pending_chunk.get(nxt, 0) > quota:
                            _pop()
            add_filler(-1, out_proj_bundles(n_jj - 1, tail=True))
            while filler:
                _pop()

    nc.compile()
    return nc


def host_consts(t=T):
    pos = np.arange(t, dtype=np.float32)[:, None]
    i = np.arange(0, D, 2, dtype=np.float32)[None, :]
    theta = pos / np.power(np.float32(10000.0), i / np.float32(D))
    cos = np.cos(theta).astype(np.float32)  # [t, 32]
    sin = np.sin(theta).astype(np.float32)
    cos2 = np.ascontiguousarray(np.tile(cos.T, (4, 1))).astype(NPBF)              # [128, t]
    sinS = np.ascontiguousarray(
        np.tile(np.concatenate([-sin.T, sin.T], 0), (2, 1))
    ).astype(NPBF)                                                                # [128, t]
    r = np.arange(P)[:, None]
    c = np.arange(P)[None, :]
    maskn = (r <= c).astype(NPBF)
    ident = np.eye(P).astype(NPBF)
    return cos2, sinS, maskn, ident


def make_in_maps(x, w_qkv, w_out):
    x = np.asarray(x, np.float32)
    w_qkv = np.asarray(w_qkv, np.float32).astype(NPBF)
    w_out = np.asarray(w_out, np.float32).astype(NPBF)
    cos2, sinS, maskn, ident = host_consts()
    in_maps = []
    for c0 in range(NCORES):
        b, g = c0 // 2, c0 % 2
        h0 = g * F
        xTb = np.ascontiguousarray(x[b].T.astype(NPBF))
        in_maps.append({
            "xT": xTb,
            "wq": np.ascontiguousarray(w_qkv[:, h0 : h0 + F]),
            "wk": np.ascontiguousarray(w_qkv[:, C + h0 : C + h0 + F]),
            "wv": np.ascontiguousarray(w_qkv[:, 2 * C + h0 : 2 * C + h0 + F]),
            "wo": np.ascontiguousarray(w_out[h0 : h0 + F, :]),
            "cos2": cos2, "sinS": sinS, "maskn": maskn, "ident": ident,
        })
    return in_maps


_REPL = {"cos2", "sinS", "maskn", "ident"}


class _Runner:
    """jit-once SPMD runner over jax.shard_map + the bass_exec custom call.

    Used instead of bass_utils.run_bass_kernel_spmd because the donation
    path in run_bass_via_pjrt hits NRT_EXEC_UNIT_UNRECOVERABLE at this
    problem size; passing non-donated zero output buffers (the kernel fully
    overwrites y) is stable. Replicating the shared inputs (rope/mask
    constants) also trims host->device traffic.
    """

    def __init__(self, nc, n_cores):
        import jax
        from jax.sharding import Mesh, PartitionSpec as PSpec
        from concourse import bass2jax

        bass2jax.install_neuronx_cc_hook()
        self.jax = jax
        self.n_cores = n_cores
        part_name = nc.partition_id_tensor.name if nc.partition_id_tensor else None
        in_names, out_names, out_avals, zero_outs = [], [], [], []
        for alloc in nc.m.functions[0].allocations:
            if not isinstance(alloc, mybir.MemoryLocationSet):
                continue
            name = alloc.memorylocations[0].name
            if alloc.kind == "ExternalInput":
                if name != part_name:
                    in_names.append(name)
            elif alloc.kind == "ExternalOutput":
                out_names.append(name)
                shape = tuple(alloc.tensor_shape)
                dtype = mybir.dt.np(alloc.dtype)
                out_avals.append(jax.core.ShapedArray(shape, dtype))
                zero_outs.append(np.zeros(shape, dtype))
        self.in_names, self.out_names = in_names, out_names
        self.out_avals, self.zero_outs = out_avals, zero_outs
        all_names = in_names + out_names + ([part_name] if part_name else [])

        def _body(*args):
            operands = list(args)
            if part_name is not None:
                operands.append(bass2jax.partition_id_tensor())
            outs = bass2jax._bass_exec_p.bind(
                *operands,
                out_avals=tuple(out_avals),
                in_names=tuple(all_names),
                out_names=tuple(out_names),
                lowering_input_output_aliases=(),
                sim_require_finite=False,
                sim_require_nnan=False,
                nc=nc,
            )
            return tuple(outs)

        try:
            from jax.experimental.shard_map import shard_map
        except ImportError:
            from jax.shard_map import shard_map
        devices = jax.devices()[:n_cores]
        self.mesh = Mesh(np.asarray(devices), ("core",))
        in_specs = tuple(
            PSpec() if nm in _REPL else PSpec("core") for nm in in_names
        ) + tuple(PSpec("core") for _ in out_names)
        out_specs = tuple(PSpec("core") for _ in out_names)
        self.fn = jax.jit(
            shard_map(_body, mesh=self.mesh, in_specs=in_specs,
                      out_specs=out_specs, check_rep=False),
            keep_unused=True,
        )

    def run(self, in_maps):
        args = []
        for nm in self.in_names:
            if nm in _REPL:
                args.append(np.asarray(in_maps[0][nm]))
            else:
                args.append(np.concatenate([np.asarray(m[nm]) for m in in_maps], axis=0))
        for z in self.zero_outs:
            args.append(np.zeros((self.n_cores * z.shape[0], *z.shape[1:]), z.dtype))
        outs = self.jax.block_until_ready(self.fn(*args))
        res = []
        for c in range(self.n_cores):
            res.append({
                nm: np.asarray(o).reshape(self.n_cores, *aval.shape)[c]
                for nm, aval, o in zip(self.out_names, self.out_avals, outs)
            })
        return res


_cache = {}


def kernel(x, w_qkv, w_out):
    if "runner" not in _cache:
        _cache["nc"] = build_nc()
        _cache["runner"] = _Runner(_cache["nc"], NCORES)
    in_maps = make_in_maps(x, w_qkv, w_out)
    results = _cache["runner"].run(in_maps)
    y = np.zeros((B, T, C), np.float32)
    for c0 in range(NCORES):
        b = c0 // 2
        y[b] += results[c0]["y"]
    return y


# revision 32
# speedup vs baseline: 1.0111x; 1.0022x over previous
import sys

sys.path.insert(0, "/opt/trn_rl_repo")

from collections import deque
from contextlib import ExitStack

import numpy as np
import ml_dtypes

import concourse.bass as bass
import concourse.tile as tile
from concourse import bacc
from concourse import mybir

B, T, C = 4, 2048, 1024
NH, D = 16, 64
NCORES = 8
# core c owns batch c//2 and head-group c%2 (8 heads = 4 pairs of 2)
NP = 4                      # head pairs per core
F = NP * 2 * D              # per-core feature slice (512)
P = 128
TCH = 512                   # token chunk for qkv phase
QCH = 512                   # q chunk in attention
KT = 128                    # k tile in attention
f32 = mybir.dt.float32
f32r = mybir.dt.float32r
bf16 = mybir.dt.bfloat16
AF = mybir.ActivationFunctionType
NPBF = ml_dtypes.bfloat16


def build_nc(reps=1):
    """One-core SPMD program: this core owns one batch and 8 heads (4 pairs)."""
    t = T
    n_cc = C // P               # contraction chunks for qkv (8)
    n_tch = t // TCH            # token chunks (4)
    n_jj = t // QCH             # q chunks (4)
    n_kt_b = t // KT            # k tiles (16)

    nc = bacc.Bacc(None, target_bir_lowering=False)
    xT = nc.declare_dram_parameter("xT", [C, t], bf16, isOutput=False)
    wq = nc.declare_dram_parameter("wq", [C, F], bf16, isOutput=False)
    wk = nc.declare_dram_parameter("wk", [C, F], bf16, isOutput=False)
    wv = nc.declare_dram_parameter("wv", [C, F], bf16, isOutput=False)
    wo = nc.declare_dram_parameter("wo", [F, C], bf16, isOutput=False)
    cos2 = nc.declare_dram_parameter("cos2", [P, t], bf16, isOutput=False)
    sinS = nc.declare_dram_parameter("sinS", [P, t], bf16, isOutput=False)
    maskn = nc.declare_dram_parameter("maskn", [P, P], bf16, isOutput=False)
    ident = nc.declare_dram_parameter("ident", [P, P], bf16, isOutput=False)
    y = nc.declare_dram_parameter("y", [t, C], f32, isOutput=True)

    xT_t = xT.rearrange("(o p) n -> p o n", p=P)   # [128, 8, t]

    with tile.TileContext(nc) as tc, ExitStack() as ctx:
        consts = ctx.enter_context(tc.tile_pool(name="consts", bufs=1))
        xpool = ctx.enter_context(tc.tile_pool(name="xt", bufs=3))
        rope = ctx.enter_context(tc.tile_pool(name="rope", bufs=3))
        ppool = ctx.enter_context(tc.tile_pool(name="pp", bufs=4))
        otp = ctx.enter_context(tc.tile_pool(name="otp", bufs=3))
        ystage = ctx.enter_context(tc.tile_pool(name="yst", bufs=3))
        ps_s = ctx.enter_context(tc.tile_pool(name="pss", bufs=2, space="PSUM"))
        ps_o = ctx.enter_context(tc.tile_pool(name="pso", bufs=1, space="PSUM"))
        ps_q = ctx.enter_context(tc.tile_pool(name="psq", bufs=2, space="PSUM"))

        # resident constants (wq first, split per-cc: the first matmul only
        # needs cc=0, so start computing after ~1/8 of the load)
        wq_r = wq.rearrange("(o p) f -> p o f", p=P)
        wq_sb = consts.tile([P, n_cc, F], bf16)
        for h in range(2):
            nc.sync.dma_start(wq_sb[:, 4 * h : 4 * h + 4], wq_r[:, 4 * h : 4 * h + 4])
        wk_r = wk.rearrange("(o p) f -> p o f", p=P)
        wk_sb = consts.tile([P, n_cc, F], bf16)
        for h in range(2):
            nc.scalar.dma_start(wk_sb[:, 4 * h : 4 * h + 4], wk_r[:, 4 * h : 4 * h + 4])
        wv_r = wv.rearrange("(o p) f -> p o f", p=P)
        wv_sb = consts.tile([P, n_cc, F], bf16)
        for h in range(2):
            nc.scalar.dma_start(wv_sb[:, 4 * h : 4 * h + 4], wv_r[:, 4 * h : 4 * h + 4])
        cos_sb = consts.tile([P, t], bf16)
        nc.scalar.dma_start(cos_sb, cos2[:, :])
        sin_sb = consts.tile([P, t], bf16)
        nc.scalar.dma_start(sin_sb, sinS[:, :])
        wo_sb = consts.tile([P, NP, C], bf16)
        nc.scalar.dma_start(wo_sb, wo.rearrange("(r p) c -> p r c", p=P))
        mask_sb = consts.tile([P, P], bf16)
        nc.scalar.dma_start(mask_sb, maskn[:, :])
        id_sb = consts.tile([P, P], bf16)
        nc.scalar.dma_start(id_sb, ident[:, :])

        # persistent per-pair tensors
        q_sb = consts.tile([P, NP, t], bf16, name="q_sb")
        k_sb = consts.tile([P, NP, t], bf16, name="k_sb")
        v1_sb = consts.tile([P, NP, n_kt_b, 2, D + 1], bf16, name="v1_sb")
        nc.vector.memset(v1_sb[:, :, :, :, D], 1.0)
        # attention output staging (all jj live): [feat, jj, pair, tok]
        ot_sb = consts.tile([P, n_jj, NP, QCH], bf16, name="ot_sb")

        H = D // 2  # 32

        def rope_apply(ps, dst, c0, bulk_pool=False):
            """dst(bf16) = ps*cos + swap32(ps)*sin, feature-major [128, TCH].

            psum reads must be DVE/ACT (GPSIMD cannot access PSUM); the final
            all-SBUF add goes to Pool to keep DVE lean."""
            cos_sl = cos_sb[:, c0 : c0 + TCH]
            sin_sl = sin_sb[:, c0 : c0 + TCH]
            raw = rope.tile([P, TCH], bf16, tag="raw")
            # chunk 0 (bulk_pool): ACT is idle before attention starts, and
            # the raw copy is what releases the qkv psum slot
            (nc.scalar.copy if bulk_pool else nc.vector.tensor_copy)(raw, ps)
            rot = rope.tile([P, TCH], bf16, tag="rot")
            for hh in range(2):
                b0 = hh * D
                nc.vector.tensor_copy(rot[b0 : b0 + H], raw[b0 + H : b0 + 2 * H])
                nc.vector.tensor_copy(rot[b0 + H : b0 + 2 * H], raw[b0 : b0 + H])
            mul_eng = nc.gpsimd if bulk_pool else nc.vector
            t1 = rope.tile([P, TCH], bf16, tag="t1")
            mul_eng.tensor_mul(out=t1, in0=raw, in1=cos_sl)
            t2 = rope.tile([P, TCH], bf16, tag="t2")
            mul_eng.tensor_mul(out=t2, in0=rot, in1=sin_sl)
            nc.gpsimd.tensor_add(out=dst, in0=t1, in1=t2)

        def qkv_chunk_bundles(it, first=False):
            """Emit-closures computing q/k/v for all pairs for token chunk it.

            All tiles are allocated lazily inside the closures so pool ring
            order matches actual emission order (bundles may be deferred)."""
            c0 = it * TCH
            bundles = []
            cell = {}
            def load(cc):
                def go():
                    if "xt" not in cell:
                        cell["xt"] = xpool.tile([P, n_cc, TCH], bf16, tag="xt", name="xt")
                    eng = nc.gpsimd if first else nc.sync
                    eng.dma_start(cell["xt"][:, cc], xT_t[:, cc, c0 : c0 + TCH])
                return go

            def load_half(h):
                # chunk 0 only: one DMA per half - fewer DGE generations at
                # the cold start
                def go():
                    if "xt" not in cell:
                        cell["xt"] = xpool.tile([P, n_cc, TCH], bf16, tag="xt", name="xt")
                    nc.gpsimd.dma_start(
                        cell["xt"][:, 4 * h : 4 * h + 4],
                        xT_t[:, 4 * h : 4 * h + 4, c0 : c0 + TCH],
                    )
                return go

            if first:
                bundles.append(load_half(0))
                bundles.append(load_half(1))
            else:
                for cc in range(n_cc):
                    bundles.append(load(cc))

            def alloc_ps(shape, key):
                # chunk 0 runs before attention: cycle its psum tiles through
                # the (idle) score pool too, doubling rotation depth
                if first:
                    n = cell.get("_pc", 0)
                    cell["_pc"] = n + 1
                    pool, tg = [(ps_q, "pq"), (ps_s, "mm")][n % 2]
                else:
                    pool, tg = ps_q, "pq"
                cell[key] = pool.tile(shape, f32, tag=tg, name=key)

            def qk_mms(w_sb, pair, key, lohi, alloc=False):
                def go():
                    if alloc:
                        alloc_ps([P, TCH], key)
                    ps = cell[key]
                    for cc in lohi:
                        nc.tensor.matmul(
                            ps,
                            lhsT=w_sb[:, cc, pair * P : (pair + 1) * P],
                            rhs=cell["xt"][:, cc, :],
                            start=(cc == 0),
                            stop=(cc == n_cc - 1),
                        )
                return go

            def rope_fin(key, dst_sb, pair):
                return lambda: rope_apply(cell[key], dst_sb[:, pair, c0 : c0 + TCH],
                                          c0, bulk_pool=first)

            def v_tile(pair, s):
                def go():
                    key = f"psv{pair}"
                    if s == 0:
                        alloc_ps([P, TCH // P, 2, D], key)
                    ps = cell[key]
                    for cc in range(n_cc):
                        nc.tensor.matmul(
                            ps[:, s],
                            lhsT=cell["xt"][:, cc, s * P : (s + 1) * P],
                            rhs=wv_sb[:, cc, pair * P : (pair + 1) * P],
                            start=(cc == 0),
                            stop=(cc == n_cc - 1),
                        )
                    ti = it * (TCH // P) + s
                    nc.vector.tensor_copy(v1_sb[:, pair, ti, :, 0:D], ps[:, s])
                return go

            for pair in range(NP):
                kq, kk = f"psq{pair}", f"psk{pair}"
                bundles.append(qk_mms(wq_sb, pair, kq, range(0, 4), alloc=True))
                bundles.append(qk_mms(wq_sb, pair, kq, range(4, 8)))
                bundles.append(rope_fin(kq, q_sb, pair))
                bundles.append(qk_mms(wk_sb, pair, kk, range(0, 4), alloc=True))
                bundles.append(qk_mms(wk_sb, pair, kk, range(4, 8)))
                bundles.append(rope_fin(kk, k_sb, pair))
                for s in range(TCH // P):
                    bundles.append(v_tile(pair, s))
            return bundles

        def out_proj_bundles(jj, tail=False):
            """Emit-closures for the out-projection of q-chunk jj (all pairs)."""
            bundles = []

            def psy_tile(qt, nh, n):
                def go():
                    if tail:
                        # attention is over: borrow the idle score pool for a
                        # wider psY rotation, and put ysb copies on both DVE
                        # and the now-idle ACT
                        pool, tg = [(ps_q, "pq"), (ps_s, "mm")][n % 2]
                        psY = pool.tile([P, 512], f32, tag=tg, name="psY")
                    else:
                        psY = ps_q.tile([P, 512], f32, tag="pq", name="psY")
                    for pair in range(NP):
                        nc.tensor.matmul(
                            psY,
                            lhsT=ot_sb[:, jj, pair, qt * P : (qt + 1) * P],
                            rhs=wo_sb[:, pair, nh * 512 : (nh + 1) * 512],
                            start=(pair == 0),
                            stop=(pair == NP - 1),
                        )
                    ysb = ystage.tile([P, 512], f32, tag="ysb")
                    if tail and n % 2 == 1:
                        nc.scalar.copy(ysb, psY)
                    else:
                        nc.vector.tensor_copy(ysb, psY)
                    r0 = jj * QCH + qt * P
                    nc.sync.dma_start(y[r0 : r0 + P, nh * 512 : (nh + 1) * 512], ysb)
                return go

            n = 0
            for qt in range(QCH // P):
                for nh in range(C // 512):
                    bundles.append(psy_tile(qt, nh, n))
                    n += 1
            return bundles

        filler = deque()          # entries: (chunk_idx_or_-1, closure)
        pending_chunk = {}
        # total fill() slots in the attention phase, for supply pacing:
        # per (jj, pair): (n_kt-1)*2 inside + 3 after the block
        pace = {"slots": sum(4 * ((4 * jj + 4 - 1) * 2 + 3) for jj in range(n_jj)),
                "acc": 0.0}

        def _pop():
            tag, fn = filler.popleft()
            if tag >= 0:
                pending_chunk[tag] -= 1
            fn()

        def fill(k):
            for _ in range(k):
                pace["slots"] -= 1
                if not filler:
                    continue
                # spread the available bundles over the remaining slots
                pace["acc"] += min(4.0, len(filler) / max(1, pace["slots"]))
                while filler and pace["acc"] >= 1.0:
                    pace["acc"] -= 1.0
                    _pop()

        def add_filler(tag, bundles):
            if tag >= 0:
                pending_chunk[tag] = pending_chunk.get(tag, 0) + len(bundles)
            filler.extend((tag, b) for b in bundles)

        def drain_chunk(cidx):
            while pending_chunk.get(cidx, 0) > 0:
                _pop()

        def emit_attn(pair, jj):
            n_kt = 4 * jj + 4
            psO = ps_o.tile([P, 2, QCH], f32, tag="o0")
            tiles = []

            def score(i):
                lo = max(0, i * KT - jj * QCH)
                qs = jj * QCH + lo
                psS = ps_s.tile([P, 2, QCH], f32, tag="mm")
                kt_sl = slice(i * KT, (i + 1) * KT)
                q_sl = slice(qs, (jj + 1) * QCH)
                for h in range(2):
                    nc.tensor.matmul(
                        psS[:, h, lo:],
                        lhsT=k_sb[h * D : (h + 1) * D, pair, kt_sl],
                        rhs=q_sb[h * D : (h + 1) * D, pair, q_sl],
                        start=True,
                        stop=True,
                    )
                Pp = ppool.tile([P, 2, QCH], bf16, tag="p0")
                nc.scalar.activation(Pp[:, :, lo:], psS[:, :, lo:], AF.Exp, scale=0.125)
                if i >= 4 * jj:  # diagonal tile: triangle mask both heads
                    nc.vector.tensor_mul(
                        Pp[:, :, lo : lo + P],
                        Pp[:, :, lo : lo + P],
                        mask_sb[:, None, :].to_broadcast((P, 2, P)),
                    )
                return lo, Pp

            def pv(i, lo, Pp):
                for h in range(2):
                    nc.tensor.matmul(
                        psO[0 : D + 1, h, lo:],
                        lhsT=v1_sb[:, pair, i, h, :],
                        rhs=Pp[:, h, lo:],
                        start=(i == 0),
                        stop=(i == n_kt - 1),
                    )

            for i in range(n_kt):
                tiles.append(score(i))
                if i >= 2:
                    # depth-2 software pipeline: PV(i-2) runs two score tiles
                    # behind, so it never waits on exp or on the previous
                    # block's psO-free chain
                    fill(2)
                    pv(i - 2, *tiles[i - 2])
            if n_kt >= 2:
                fill(1)
                pv(n_kt - 2, *tiles[n_kt - 2])
            pv(n_kt - 1, *tiles[n_kt - 1])

            # Copy psO out first (DVE: GPSIMD cannot read PSUM): frees the
            # psum bank for the next (pair, jj) block earlier than the
            # recip/mul chain would.
            oc = otp.tile([D + 1, 2, QCH], f32, tag="oc")
            nc.vector.tensor_copy(oc, psO[0 : D + 1, :, :])
            # softmax normalization: OT[h] = O[h] / l[h]; muls on Pool
            rs = otp.tile([1, 2, QCH], f32, tag="rs")
            nc.vector.reciprocal(rs, oc[D : D + 1, :, :])
            rb0 = otp.tile([D, QCH], f32, tag="rb0")
            nc.gpsimd.partition_broadcast(rb0, rs[0:1, 0, :])
            rb1 = otp.tile([D, QCH], f32, tag="rb1")
            nc.gpsimd.partition_broadcast(rb1, rs[0:1, 1, :])
            if jj == n_jj - 1 and pair == NP - 1:
                for qt in range(QCH // P):
                    qsl = slice(qt * P, (qt + 1) * P)
                    nc.vector.tensor_mul(ot_sb[0:D, jj, pair, qsl], oc[0:D, 0, qsl], rb0[:, qsl])
                    nc.vector.tensor_mul(ot_sb[D:P, jj, pair, qsl], oc[0:D, 1, qsl], rb1[:, qsl])
            else:
                nc.gpsimd.tensor_mul(ot_sb[0:D, jj, pair, :], oc[0:D, 0, :], rb0)
                nc.gpsimd.tensor_mul(ot_sb[D:P, jj, pair, :], oc[0:D, 1, :], rb1)

        # ---- emission schedule ----
        for rep in range(reps):
            # chunk 0 emitted directly; chunk jj+1 rides as filler inside
            # attn(*, jj) so attention (and ACT exp work) starts early
            for b in qkv_chunk_bundles(0, first=(rep == 0)):
                b()
            for jj in range(n_jj):
                # qkv chunk jj must be fully emitted before attn(*, jj)
                drain_chunk(jj)
                if jj + 1 < n_tch:
                    add_filler(jj + 1, qkv_chunk_bundles(jj + 1))
                if jj == n_jj - 1:
                    # out-proj of jj 0..2 feeds PE during the filler-starved
                    # final attention sweep
                    for j2 in range(n_jj - 1):
                        add_filler(-1, out_proj_bundles(j2))
                nxt = jj + 1 if jj + 1 < n_tch else None
                quota0 = pending_chunk.get(nxt, 0) if nxt is not None else 0
                for pair in range(NP):
                    emit_attn(pair, jj)
                    fill(3)
                    if nxt is not None:
                        quota = (NP - 1 - pair) * quota0 // NP
                        while pending_chunk.get(nxt, 0) > quota:
                            _pop()
            add_filler(-1, out_proj_bundles(n_jj - 1, tail=True))
            while filler:
                _pop()

    nc.compile()
    return nc


def host_consts(t=T):
    pos = np.arange(t, dtype=np.float32)[:, None]
    i = np.arange(0, D, 2, dtype=np.float32)[None, :]
    theta = pos / np.power(np.float32(10000.0), i / np.float32(D))
    cos = np.cos(theta).astype(np.float32)  # [t, 32]
    sin = np.sin(theta).astype(np.float32)
    cos2 = np.ascontiguousarray(np.tile(cos.T, (4, 1))).astype(NPBF)              # [128, t]
    sinS = np.ascontiguousarray(
        np.tile(np.concatenate([-sin.T, sin.T], 0), (2, 1))
    ).astype(NPBF)                                                                # [128, t]
    r = np.arange(P)[:, None]
    c = np.arange(P)[None, :]
    maskn = (r <= c).astype(NPBF)
    ident = np.eye(P).astype(NPBF)
    return cos2, sinS, maskn, ident


def make_in_maps(x, w_qkv, w_out):
    x = np.asarray(x, np.float32)
    w_qkv = np.asarray(w_qkv, np.float32).astype(NPBF)
    w_out = np.asarray(w_out, np.float32).astype(NPBF)
    cos2, sinS, maskn, ident = host_consts()
    in_maps = []
    for c0 in range(NCORES):
        b, g = c0 // 2, c0 % 2
        h0 = g * F
        xTb = np.ascontiguousarray(x[b].T.astype(NPBF))
        in_maps.append({
            "xT": xTb,
            "wq": np.ascontiguousarray(w_qkv[:, h0 : h0 + F]),
            "wk": np.ascontiguousarray(w_qkv[:, C + h0 : C + h0 + F]),
            "wv": np.ascontiguousarray(w_qkv[:, 2 * C + h0 : 2 * C + h0 + F]),
            "wo": np.ascontiguousarray(w_out[h0 : h0 + F, :]),
            "cos2": cos2, "sinS": sinS, "maskn": maskn, "ident": ident,
        })
    return in_maps


_REPL = {"cos2", "sinS", "maskn", "ident"}


class _Runner:
    """jit-once SPMD runner over jax.shard_map + the bass_exec custom call.

    Used instead of bass_utils.run_bass_kernel_spmd because the donation
    path in run_bass_via_pjrt hits NRT_EXEC_UNIT_UNRECOVERABLE at this
    problem size; passing non-donated zero output buffers (the kernel fully
    overwrites y) is stable. Replicating the shared inputs (rope/mask
    constants) also trims host->device traffic.
    """

    def __init__(self, nc, n_cores):
        import jax
        from jax.sharding import Mesh, PartitionSpec as PSpec
        from concourse import bass2jax

        bass2jax.install_neuronx_cc_hook()
        self.jax = jax
        self.n_cores = n_cores
        part_name = nc.partition_id_tensor.name if nc.partition_id_tensor else None
        in_names, out_names, out_avals, zero_outs = [], [], [], []
        for alloc in nc.m.functions[0].allocations:
            if not isinstance(alloc, mybir.MemoryLocationSet):
                continue
            name = alloc.memorylocations[0].name
            if alloc.kind == "ExternalInput":
                if name != part_name:
                    in_names.append(name)
            elif alloc.kind == "ExternalOutput":
                out_names.append(name)
                shape = tuple(alloc.tensor_shape)
                dtype = mybir.dt.np(alloc.dtype)
                out_avals.append(jax.core.ShapedArray(shape, dtype))
                zero_outs.append(np.zeros(shape, dtype))
        self.in_names, self.out_names = in_names, out_names
        self.out_avals, self.zero_outs = out_avals, zero_outs
        all_names = in_names + out_names + ([part_name] if part_name else [])

        def _body(*args):
            operands = list(args)
            if part_name is not None:
                operands.append(bass2jax.partition_id_tensor())
            outs = bass2jax._bass_exec_p.bind(
                *operands,
                out_avals=tuple(out_avals),
                in_names=tuple(all_names),
                out_names=tuple(out_names),
                lowering_input_output_aliases=(),
                sim_require_finite=False,
                sim_require_nnan=False,
                nc=nc,
            )
            return tuple(outs)

        try:
            from jax.experimental.shard_map import shard_map
        except ImportError:
            from jax.shard_map import shard_map
        devices = jax.devices()[:n_cores]
        self.mesh = Mesh(np.asarray(devices), ("core",))
        in_specs = tuple(
            PSpec() if nm in _REPL else PSpec("core") for nm in in_names
        ) + tuple(PSpec("core") for _ in out_names)
        out_specs = tuple(PSpec("core") for _ in out_names)
        self.fn = jax.jit(
            shard_map(_body, mesh=self.mesh, in_specs=in_specs,
                      out_specs=out_specs, check_rep=False),
            keep_unused=True,
        )

    def run(self, in_maps):
        args = []
        for nm in self.in_names:
            if nm in _REPL:
                args.append(np.asarray(in_maps[0][nm]))
            else:
                args.append(np.concatenate([np.asarray(m[nm]) for m in in_maps], axis=0))
        for z in self.zero_outs:
            args.append(np.zeros((self.n_cores * z.shape[0], *z.shape[1:]), z.dtype))
        outs = self.jax.block_until_ready(self.fn(*args))
        res = []
        for c in range(self.n_cores):
            res.append({
                nm: np.asarray(o).reshape(self.n_cores, *aval.shape)[c]
                for nm, aval, o in zip(self.out_names, self.out_avals, outs)
            })
        return res


_cache = {}


def kernel(x, w_qkv, w_out):
    if "runner" not in _cache:
        _cache["nc"] = build_nc()
        _cache["runner"] = _Runner(_cache["nc"], NCORES)
    in_maps = make_in_maps(x, w_qkv, w_out)
    results = _cache["runner"].run(in_maps)
    y = np.zeros((B, T, C), np.float32)
    for c0 in range(NCORES):
        b = c0 // 2
        y[b] += results[c0]["y"]
    return y
